# revision 7
# baseline (speedup 1.0000x reference)
"""HTSK fuzzy-system kernel for Trainium2 (Bass/Tile), 8-core data-parallel.

Math (per batch row b):
  S     = H/sigma^2 + EPS                          (D,R)
  m     = mean_d(-(X_bd - C_dr)^2 * S_dr)          (B,R)
        = X^2 @ (-S/D) + X @ (2*S*C/D) + K2        (matmul expansion)
  e     = exp(m - max_r m) / sum_r exp(...)        (normalized firing)
  out   = sum_r e_br * G_bro  +  e @ (W2 + 1 b^T)

Transposed formulation: G^T[o*R+r, b] = sum_d Wt[d, o*R+r] X^T[d, b] is
computed in 64 o-chunks of [128r, 512b]; each chunk is multiplied
elementwise by e^T (flat 2-tensor TT on DVE, 2x mode) and the r-sum is
done on the PE with a one-hot selection matmul accumulating out^T[o, b]
into a single PSUM bank. This moves the whole r-reduction tree off DVE.

Sharding: batch B=4096 split 512 rows per core; weights replicated.
"""
import sys
import types
from contextlib import ExitStack

import numpy as np

sys.path.insert(0, "/opt/trn_rl_repo")

# NTFF profile-hook registry: trn_boot §6 sets it at jax init, concourse
# bass_utils reads it when trace=True. The container's antenv package lacks
# this submodule, so provide it before anything imports jax/concourse.
if "antenv.axon_hooks" not in sys.modules:
    _ah = types.ModuleType("antenv.axon_hooks")
    _ah._hook = None

    def _set_hook(hook):
        _ah._hook = hook

    def _get_hook():
        return _ah._hook

    _ah.set_axon_ntff_profile_hook = _set_hook
    _ah.get_axon_ntff_profile_hook = _get_hook
    sys.modules["antenv.axon_hooks"] = _ah

import ml_dtypes  # noqa: E402
import concourse.bass as bass  # noqa: E402
import concourse.bacc as bacc  # noqa: E402
import concourse.tile as tile  # noqa: E402
from concourse import mybir  # noqa: E402
from concourse import bass_utils  # noqa: E402
from concourse.masks import make_identity  # noqa: E402

H = 0.5
EPS = 1e-8
B, D, R, O = 4096, 256, 128, 64
NCORES = 8
BL = B // NCORES          # 512 batch rows per core
NT = BL // 128            # 4 partition tiles per core
RO = R * O                # 8192
SLAG = 2                  # S-matmul chunk lag behind its G-matmul
F32 = mybir.dt.float32
BF16 = mybir.dt.bfloat16

_CACHE = {}


def _build():
    nc = bacc.Bacc("TRN2", target_bir_lowering=False, debug=False)
    X = nc.dram_tensor("X", [BL, D], F32, kind="ExternalInput")
    A = nc.dram_tensor("A", [D, R], F32, kind="ExternalInput")
    Bm = nc.dram_tensor("Bm", [D, R], F32, kind="ExternalInput")
    K2 = nc.dram_tensor("K2", [1, R], F32, kind="ExternalInput")
    W2p = nc.dram_tensor("W2p", [R, O], BF16, kind="ExternalInput")
    Wt = nc.dram_tensor("Wt", [D, RO], BF16, kind="ExternalInput")
    out = nc.dram_tensor("out", [BL, O], F32, kind="ExternalOutput")

    with tile.TileContext(nc) as tc, ExitStack() as ctx:
        consts = ctx.enter_context(tc.tile_pool(name="consts", bufs=1))
        work = ctx.enter_context(tc.tile_pool(name="work", bufs=2))
        gsbp = ctx.enter_context(tc.tile_pool(name="gsb", bufs=4))
        ps_x = ctx.enter_context(tc.tile_pool(name="ps_x", bufs=1, space="PSUM"))
        ps_m = ctx.enter_context(tc.tile_pool(name="ps_m", bufs=1, space="PSUM"))
        ps_e = ctx.enter_context(tc.tile_pool(name="ps_e", bufs=1, space="PSUM"))
        ps_o = ctx.enter_context(tc.tile_pool(name="ps_o", bufs=1, space="PSUM"))
        ps_g = ctx.enter_context(tc.tile_pool(name="ps_g", bufs=3, space="PSUM"))

        # ---- constants (loaded once); small ones first so compute can start,
        # Wt streams on SWDGE queues leaving HWDGE free for X tiles ----
        a_sb = consts.tile([128, 2, R], F32, tag="a")
        bm_sb = consts.tile([128, 2, R], F32, tag="bm")
        for c in range(2):
            nc.sync.dma_start(out=a_sb[:, c, :], in_=A[c * 128:(c + 1) * 128, :])
            nc.sync.dma_start(out=bm_sb[:, c, :], in_=Bm[c * 128:(c + 1) * 128, :])
        k2_sb = consts.tile([1, R], F32, tag="k2")
        nc.sync.dma_start(out=k2_sb[:, :], in_=K2[:, :])
        w2p_sb = consts.tile([R, O], BF16, tag="w2p")
        nc.sync.dma_start(out=w2p_sb[:, :], in_=W2p[:, :])
        identF = consts.tile([128, 128], F32, tag="idf")
        make_identity(nc, identF)
        identB = consts.tile([128, 128], BF16, tag="idb")
        make_identity(nc, identB)
        ones_sb = consts.tile([1, 128], F32, tag="ones")
        nc.vector.memset(ones_sb, 1.0)
        # Z: zeros except column 64 all-ones; Z[:, 64-o : 128-o] is the
        # [128, 64] one-hot selector whose column o is all-ones.
        z_sb = consts.tile([128, 128], BF16, tag="z")
        nc.vector.memset(z_sb, 0.0)
        nc.vector.memset(z_sb[:, 64:65], 1.0)
        wt_sb = []  # wt_sb[c][q]: (128, 2048) bf16, d-chunk c, column quarter q
        for c in range(2):
            row = []
            for q in range(4):
                t_ = consts.tile([128, 2048], BF16, tag=f"wt{c}{q}")
                nc.gpsimd.dma_start(
                    out=t_[:, :], in_=Wt[c * 128:(c + 1) * 128, q * 2048:(q + 1) * 2048]
                )
                row.append(t_)
            wt_sb.append(row)

        # X^T (bf16, for G rhs) and e^T for all 512 batch rows
        xTb_all = [
            consts.tile([128, BL], BF16, tag=f"xtb{c}", name=f"xtb{c}")
            for c in range(2)
        ]
        eT_all = consts.tile([128, BL], BF16, tag="eT")

        # ---- prologue: per b-tile transpose, membership, softmax ----
        for t in range(NT):
            bs = slice(t * 128, (t + 1) * 128)
            xt = work.tile([128, D], F32, tag="xt")
            nc.sync.dma_start(out=xt[:, :], in_=X[bs, :])
            xtT_ps = ps_x.tile([128, D], F32, tag="xtT")
            for c in range(2):
                nc.tensor.transpose(
                    xtT_ps[:, c * 128:(c + 1) * 128],
                    xt[:, c * 128:(c + 1) * 128],
                    identF,
                )
            xT = work.tile([128, D], F32, tag="xT")
            x2T = work.tile([128, D], F32, tag="x2T")
            nc.scalar.copy(xT, xtT_ps)
            nc.scalar.activation(x2T, xtT_ps, mybir.ActivationFunctionType.Square)
            for c in range(2):
                nc.vector.tensor_copy(
                    xTb_all[c][:, bs], xtT_ps[:, c * 128:(c + 1) * 128]
                )

            # m = X^2 @ A + X @ Bm + 1*K2   (b-partition layout for softmax)
            m_ps = ps_m.tile([128, R], F32, tag="m")
            nc.tensor.matmul(m_ps, lhsT=x2T[:, 0:128], rhs=a_sb[:, 0, :],
                             start=True, stop=False)
            nc.tensor.matmul(m_ps, lhsT=x2T[:, 128:256], rhs=a_sb[:, 1, :],
                             start=False, stop=False)
            nc.tensor.matmul(m_ps, lhsT=xT[:, 0:128], rhs=bm_sb[:, 0, :],
                             start=False, stop=False)
            nc.tensor.matmul(m_ps, lhsT=xT[:, 128:256], rhs=bm_sb[:, 1, :],
                             start=False, stop=False)
            nc.tensor.matmul(m_ps, lhsT=ones_sb, rhs=k2_sb,
                             start=False, stop=True)

            # normalized softmax e_n = exp(m - max)/sum, then e_n^T
            nmx = work.tile([128, 1], F32, tag="nmx")
            nc.vector.reduce_max(nmx, m_ps, axis=mybir.AxisListType.X, negate=True)
            e_bf = work.tile([128, R], BF16, tag="e")
            s_ = work.tile([128, 1], F32, tag="s")
            nc.scalar.activation(e_bf, m_ps, mybir.ActivationFunctionType.Exp,
                                 bias=nmx, scale=1.0, accum_out=s_)
            rs = work.tile([128, 1], F32, tag="rs")
            nc.vector.reciprocal(rs, s_)
            e_n = work.tile([128, R], BF16, tag="en")
            nc.vector.tensor_scalar_mul(e_n, e_bf, rs)
            eT_ps = ps_e.tile([128, 128], BF16, tag="eTp")
            nc.tensor.transpose(eT_ps, e_n, identB)
            nc.scalar.copy(eT_all[:, bs], eT_ps)

        # ---- main loop: G^T chunks, e-multiply, one-hot reduction ----
        out_ps = ps_o.tile([64, BL], F32, tag="outT")
        nc.tensor.matmul(out_ps, lhsT=w2p_sb, rhs=eT_all, start=True, stop=False)

        gsb_q = []

        def s_matmul(o, gsb, last):
            nc.tensor.matmul(out_ps, lhsT=z_sb[:, 64 - o:128 - o], rhs=gsb,
                             start=False, stop=last)

        for o in range(64):
            q, col = divmod(o * 128, 2048)
            gps = ps_g.tile([128, BL], F32, tag="g", name=f"g_{o}")
            for c in range(2):
                nc.tensor.matmul(
                    gps,
                    lhsT=wt_sb[c][q][:, col:col + 128],
                    rhs=xTb_all[c],
                    start=(c == 0), stop=(c == 1),
                )
            gsb = gsbp.tile([128, BL], BF16, tag="gsb", name=f"gsb_{o}")
            if o % 4 == 3:
                nc.vector.tensor_copy(gsb, gps)
            else:
                nc.scalar.copy(gsb, gps)
            nc.vector.tensor_mul(gsb, gsb, eT_all)
            gsb_q.append(gsb)
            if o >= SLAG:
                s_matmul(o - SLAG, gsb_q[o - SLAG], last=False)
        for o in range(64 - SLAG, 64):
            s_matmul(o, gsb_q[o], last=(o == 63))

        # ---- final: out^T -> out tiles, DMA ----
        outT_sb = consts.tile([64, BL], F32, tag="outsb")
        nc.scalar.copy(outT_sb, out_ps)
        for t in range(NT):
            bs = slice(t * 128, (t + 1) * 128)
            o_ps = ps_x.tile([128, O], F32, tag="ops", name=f"ops_{t}")
            nc.tensor.transpose(o_ps, outT_sb[:, bs], identF[0:64, 0:64])
            osb = work.tile([128, O], F32, tag="osb", name=f"osb_{t}")
            nc.vector.tensor_copy(osb, o_ps)
            nc.sync.dma_start(out=out[bs, :], in_=osb)

    nc.finalize()
    return nc


def _get_nc():
    if "nc" not in _CACHE:
        _CACHE["nc"] = _build()
    return _CACHE["nc"]


def _host_prep(centers, sigmas, W, b):
    c64 = centers.astype(np.float64)
    S = (H / sigmas.astype(np.float64) ** 2) + EPS          # (D,R)
    A = (-S / D).astype(np.float32)                          # X^2 coeff
    Bm = (2.0 * S * c64 / D).astype(np.float32)              # X coeff
    K2 = (-(S * c64 * c64).sum(axis=0, keepdims=True) / D).astype(np.float32)
    W1 = W[: D * R].reshape(R, D, O)
    # o-major: Wt[d, o*R + r] = W1[r, d, o]
    Wt = np.ascontiguousarray(W1.transpose(1, 2, 0).reshape(D, RO)).astype(
        ml_dtypes.bfloat16
    )
    W2p = (W[D * R:].astype(np.float64) + b[None, :].astype(np.float64)).astype(
        ml_dtypes.bfloat16
    )
    return A, Bm, K2, W2p, Wt


def kernel(X, centers, sigmas, W, b):
    X = np.asarray(X, dtype=np.float32)
    centers = np.asarray(centers, dtype=np.float32)
    sigmas = np.asarray(sigmas, dtype=np.float32)
    W = np.asarray(W, dtype=np.float32)
    b = np.asarray(b, dtype=np.float32)

    A, Bm, K2, W2p, Wt = _host_prep(centers, sigmas, W, b)
    nc = _get_nc()
    in_maps = [
        {
            "X": np.ascontiguousarray(X[k * BL:(k + 1) * BL]),
            "A": A, "Bm": Bm, "K2": K2, "W2p": W2p, "Wt": Wt,
        }
        for k in range(NCORES)
    ]
    res = bass_utils.run_bass_kernel_spmd(nc, in_maps, core_ids=list(range(NCORES)))
    return np.concatenate([res.results[k]["out"] for k in range(NCORES)], axis=0)


# revision 8
# speedup vs baseline: 1.2090x; 1.2090x over previous
"""HTSK fuzzy-system kernel for Trainium2 (Bass/Tile), 8-core data-parallel.

Math (per batch row b):
  S     = H/sigma^2 + EPS                          (D,R)
  m     = mean_d(-(X_bd - C_dr)^2 * S_dr)          (B,R)
        = X^2 @ (-S/D) + X @ (2*S*C/D) + K2        (matmul expansion)
  e     = exp(m - max_r m) / sum_r exp(...)        (normalized firing)
  out   = sum_r e_br * G_bro  +  e @ (W2 + 1 b^T)
  G     = X @ Wt,  Wt[d, o*R+r] = W[r*D+d, o]      (B, O*R)  o-major

o-major G columns make the firing-strength multiply read e with a
contiguous innermost r-run (DVE 2x mode) and give the r-reduction tree
strided-outer / flat-write access patterns that also keep 2x mode.
G is processed in two o-halves per tile so the DVE multiply+tree of one
half overlaps the Scalar PSUM evictions of the other.

Sharding: batch B=4096 split 512 rows per core; weights replicated.
"""
import sys
import types
from contextlib import ExitStack

import numpy as np

sys.path.insert(0, "/opt/trn_rl_repo")

# NTFF profile-hook registry: trn_boot §6 sets it at jax init, concourse
# bass_utils reads it when trace=True. The container's antenv package lacks
# this submodule, so provide it before anything imports jax/concourse.
if "antenv.axon_hooks" not in sys.modules:
    _ah = types.ModuleType("antenv.axon_hooks")
    _ah._hook = None

    def _set_hook(hook):
        _ah._hook = hook

    def _get_hook():
        return _ah._hook

    _ah.set_axon_ntff_profile_hook = _set_hook
    _ah.get_axon_ntff_profile_hook = _get_hook
    sys.modules["antenv.axon_hooks"] = _ah

import ml_dtypes  # noqa: E402
import concourse.bass as bass  # noqa: E402
import concourse.bacc as bacc  # noqa: E402
import concourse.tile as tile  # noqa: E402
from concourse import mybir  # noqa: E402
from concourse import bass_utils  # noqa: E402
from concourse.masks import make_identity  # noqa: E402

H = 0.5
EPS = 1e-8
B, D, R, O = 4096, 256, 128, 64
NCORES = 8
BL = B // NCORES          # 512 batch rows per core
NT = BL // 128            # 4 partition tiles per core
RO = R * O                # 8192
HO = O // 2               # 32 o's per half
HW = HO * R               # 4096 cols per half
F32 = mybir.dt.float32
BF16 = mybir.dt.bfloat16

_CACHE = {}


def _build():
    nc = bacc.Bacc("TRN2", target_bir_lowering=False, debug=False)
    X = nc.dram_tensor("X", [BL, D], F32, kind="ExternalInput")
    A = nc.dram_tensor("A", [D, R], BF16, kind="ExternalInput")
    Bm = nc.dram_tensor("Bm", [D, R], BF16, kind="ExternalInput")
    K2 = nc.dram_tensor("K2", [1, R], F32, kind="ExternalInput")
    W2p = nc.dram_tensor("W2p", [R, O], BF16, kind="ExternalInput")
    Wt = nc.dram_tensor("Wt", [D, RO], BF16, kind="ExternalInput")
    out = nc.dram_tensor("out", [BL, O], F32, kind="ExternalOutput")

    with tile.TileContext(nc) as tc, ExitStack() as ctx:
        consts = ctx.enter_context(tc.tile_pool(name="consts", bufs=1))
        xtp = ctx.enter_context(tc.tile_pool(name="xtp", bufs=4))
        work = ctx.enter_context(tc.tile_pool(name="work", bufs=2))
        gmp = ctx.enter_context(tc.tile_pool(name="gm", bufs=2))
        treep = ctx.enter_context(tc.tile_pool(name="tree", bufs=2))
        ps_x = ctx.enter_context(tc.tile_pool(name="ps_x", bufs=1, space="PSUM"))
        ps_m = ctx.enter_context(tc.tile_pool(name="ps_m", bufs=1, space="PSUM"))
        ps_e = ctx.enter_context(tc.tile_pool(name="ps_e", bufs=1, space="PSUM"))
        ps_o = ctx.enter_context(tc.tile_pool(name="ps_o", bufs=1, space="PSUM"))
        ps_g = ctx.enter_context(tc.tile_pool(name="ps_g", bufs=2, space="PSUM"))

        # ---- X tiles first on the sync queue (small, unblocks compute),
        # then the rest of the constants; Wt streams on SWDGE queues ----
        xts = []
        for t in range(NT):
            xt = xtp.tile([128, D], F32, tag="xt", name=f"xt{t}")
            nc.sync.dma_start(out=xt[:, :], in_=X[t * 128:(t + 1) * 128, :])
            xts.append(xt)
        a_sb = consts.tile([128, 2, R], BF16, tag="a")
        bm_sb = consts.tile([128, 2, R], BF16, tag="bm")
        for c in range(2):
            nc.sync.dma_start(out=a_sb[:, c, :], in_=A[c * 128:(c + 1) * 128, :])
            nc.sync.dma_start(out=bm_sb[:, c, :], in_=Bm[c * 128:(c + 1) * 128, :])
        k2_sb = consts.tile([1, R], F32, tag="k2")
        nc.sync.dma_start(out=k2_sb[:, :], in_=K2[:, :])
        w2p_sb = consts.tile([R, O], BF16, tag="w2p")
        nc.sync.dma_start(out=w2p_sb[:, :], in_=W2p[:, :])
        identF = consts.tile([128, 128], F32, tag="idf")
        make_identity(nc, identF)
        identB = consts.tile([128, 128], BF16, tag="idb")
        make_identity(nc, identB)
        ones_sb = consts.tile([1, 128], F32, tag="ones")
        nc.vector.memset(ones_sb, 1.0)
        wt_sb = []  # wt_sb[c][q]: (128, 2048) bf16, d-chunk c, column quarter q
        for c in range(2):
            row = []
            for q in range(4):
                t_ = consts.tile([128, 2048], BF16, tag=f"wt{c}{q}")
                nc.gpsimd.dma_start(
                    out=t_[:, :], in_=Wt[c * 128:(c + 1) * 128, q * 2048:(q + 1) * 2048]
                )
                row.append(t_)
            wt_sb.append(row)

        for t in range(NT):
            # ---- transpose X tile (b,d) -> (d,b); bf16 + squared bf16 ----
            xt = xts[t]
            xtT_ps = ps_x.tile([128, D], F32, tag="xtT")
            for c in range(2):
                nc.tensor.transpose(
                    xtT_ps[:, c * 128:(c + 1) * 128],
                    xt[:, c * 128:(c + 1) * 128],
                    identF,
                )
            x2b = work.tile([128, D], BF16, tag="x2b")
            xTb = work.tile([128, D], BF16, tag="xTb")
            nc.scalar.activation(x2b, xtT_ps, mybir.ActivationFunctionType.Square)
            nc.vector.tensor_copy(xTb, xtT_ps)

            # ---- membership logits m = X^2 @ A + X @ Bm + 1*K2 ----
            m_ps = ps_m.tile([128, R], F32, tag="m")
            nc.tensor.matmul(m_ps, lhsT=x2b[:, 0:128], rhs=a_sb[:, 0, :],
                             start=True, stop=False)
            nc.tensor.matmul(m_ps, lhsT=x2b[:, 128:256], rhs=a_sb[:, 1, :],
                             start=False, stop=False)
            nc.tensor.matmul(m_ps, lhsT=xTb[:, 0:128], rhs=bm_sb[:, 0, :],
                             start=False, stop=False)
            nc.tensor.matmul(m_ps, lhsT=xTb[:, 128:256], rhs=bm_sb[:, 1, :],
                             start=False, stop=False)
            nc.tensor.matmul(m_ps, lhsT=ones_sb, rhs=k2_sb,
                             start=False, stop=True)

            # ---- normalized softmax: e_n = exp(m - max) / sum ----
            nmx = work.tile([128, 1], F32, tag="nmx")
            nc.vector.reduce_max(nmx, m_ps, axis=mybir.AxisListType.X, negate=True)
            e_bf = work.tile([128, R], BF16, tag="e")
            s_ = work.tile([128, 1], F32, tag="s")
            nc.scalar.activation(e_bf, m_ps, mybir.ActivationFunctionType.Exp,
                                 bias=nmx, scale=1.0, accum_out=s_)
            rs = work.tile([128, 1], F32, tag="rs")
            nc.vector.reciprocal(rs, s_)
            e_n = work.tile([128, R], BF16, tag="en")
            nc.vector.tensor_scalar_mul(e_n, e_bf, rs)

            # ---- out2 = e_n @ W2p  (needs e_n^T as stationary) ----
            eT_ps = ps_e.tile([128, 128], BF16, tag="eT")
            nc.tensor.transpose(eT_ps, e_n, identB)
            eT_sb = work.tile([128, 128], BF16, tag="eTsb")
            nc.scalar.copy(eT_sb, eT_ps)
            out2_ps = ps_o.tile([128, O], F32, tag="out2")
            nc.tensor.matmul(out2_ps, lhsT=eT_sb, rhs=w2p_sb, start=True, stop=True)

            # ---- G = X @ Wt in two o-halves; per half: 4x 1024-col PSUM
            # chunks (Scalar evicts), one in-place e-multiply, halving tree ----
            osb = work.tile([128, O], F32, tag="osb")
            for h in range(2):
                gm = gmp.tile([128, HW], BF16, tag=f"gm{h}", name=f"gm_{t}_{h}")
                for hc in range(4):
                    ch = h * 4 + hc
                    gt = ps_g.tile([128, 1024], F32, tag="g", name=f"g_{t}_{ch}")
                    for c in range(2):
                        for half in range(2):
                            nch = 2 * ch + half
                            q, col = divmod(nch * 512, 2048)
                            nc.tensor.matmul(
                                gt[:, half * 512:(half + 1) * 512],
                                lhsT=xTb[:, c * 128:(c + 1) * 128],
                                rhs=wt_sb[c][q][:, col:col + 512],
                                start=(c == 0), stop=(c == 1),
                            )
                    nc.scalar.copy(gm[:, hc * 1024:(hc + 1) * 1024], gt)

                # firing-strength multiply in place over the half
                gv = gm.rearrange("p (o r) -> p o r", o=HO)
                ebc = e_n.rearrange("p r -> p () r").broadcast_to((128, HO, R))
                nc.vector.tensor_mul(gv, gv, ebc)

                # weighted sum over r: halving tree per o-block (all DVE)
                prev = gm
                r = R
                while r > 2:
                    nxt = treep.tile([128, HO * (r // 2)], BF16,
                                     tag=f"tr{h}{r}", name=f"tr_{t}_{h}_{r}")
                    pv = prev.rearrange("p (o r) -> p o r", o=HO)
                    nv = nxt.rearrange("p (o r) -> p o r", o=HO)
                    nc.vector.tensor_add(nv, pv[:, :, 0:r // 2], pv[:, :, r // 2:r])
                    prev = nxt
                    r //= 2
                pv = prev.rearrange("p (o r) -> p o r", o=HO)
                hsl = slice(h * HO, (h + 1) * HO)
                # out = tree + out2 for this o-half (firing already normalized)
                red = work.tile([128, HO], F32, tag=f"red{h}", name=f"red_{t}_{h}")
                nc.vector.tensor_add(red.rearrange("p o -> p o ()"),
                                     pv[:, :, 0:1], pv[:, :, 1:2])
                nc.vector.tensor_add(osb[:, hsl], red, out2_ps[:, hsl])
            nc.sync.dma_start(out=out[t * 128:(t + 1) * 128, :], in_=osb)

    nc.finalize()
    return nc


def _get_nc():
    if "nc" not in _CACHE:
        _CACHE["nc"] = _build()
    return _CACHE["nc"]


def _host_prep(centers, sigmas, W, b):
    c64 = centers.astype(np.float64)
    S = (H / sigmas.astype(np.float64) ** 2) + EPS          # (D,R)
    A = (-S / D).astype(ml_dtypes.bfloat16)                  # X^2 coeff
    Bm = (2.0 * S * c64 / D).astype(ml_dtypes.bfloat16)      # X coeff
    K2 = (-(S * c64 * c64).sum(axis=0, keepdims=True) / D).astype(np.float32)
    W1 = W[: D * R].reshape(R, D, O)
    # o-major: Wt[d, o*R + r] = W1[r, d, o]
    Wt = np.ascontiguousarray(W1.transpose(1, 2, 0).reshape(D, RO)).astype(
        ml_dtypes.bfloat16
    )
    W2p = (W[D * R:].astype(np.float64) + b[None, :].astype(np.float64)).astype(
        ml_dtypes.bfloat16
    )
    return A, Bm, K2, W2p, Wt


def kernel(X, centers, sigmas, W, b):
    X = np.asarray(X, dtype=np.float32)
    centers = np.asarray(centers, dtype=np.float32)
    sigmas = np.asarray(sigmas, dtype=np.float32)
    W = np.asarray(W, dtype=np.float32)
    b = np.asarray(b, dtype=np.float32)

    A, Bm, K2, W2p, Wt = _host_prep(centers, sigmas, W, b)
    nc = _get_nc()
    in_maps = [
        {
            "X": np.ascontiguousarray(X[k * BL:(k + 1) * BL]),
            "A": A, "Bm": Bm, "K2": K2, "W2p": W2p, "Wt": Wt,
        }
        for k in range(NCORES)
    ]
    res = bass_utils.run_bass_kernel_spmd(nc, in_maps, core_ids=list(range(NCORES)))
    return np.concatenate([res.results[k]["out"] for k in range(NCORES)], axis=0)


# revision 9
# speedup vs baseline: 1.2767x; 1.0560x over previous
"""HTSK fuzzy-system kernel for Trainium2 (Bass/Tile), 8-core data-parallel.

Math (per batch row b):
  S     = H/sigma^2 + EPS                          (D,R)
  m     = mean_d(-(X_bd - C_dr)^2 * S_dr)          (B,R)
        = X^2 @ (-S/D) + X @ (2*S*C/D) + K2        (matmul expansion)
  e     = exp(m - max_r m) / sum_r exp(...)        (normalized firing)
  out   = sum_r e_br * G_bro  +  e @ (W2 + 1 b^T)
  G     = X @ Wt,  Wt[d, o*R+r] = W[r*D+d, o]      (B, O*R)  o-major

o-major G columns make the firing-strength multiply read e with a
contiguous innermost r-run (DVE 2x mode) and give the r-reduction tree
strided-outer / flat-write access patterns that also keep 2x mode.

Schedule: all four per-tile prologues (transpose, membership, softmax,
out2) run first — they only need the small DMAs — hiding the ~12 us Wt
stream; then the four G phases run back-to-back (PE dense, Scalar
evicts PSUM, DVE multiplies + reduces, in two o-halves per tile).

Sharding: batch B=4096 split 512 rows per core; weights replicated.
"""
import sys
import types
from contextlib import ExitStack

import numpy as np

sys.path.insert(0, "/opt/trn_rl_repo")

# NTFF profile-hook registry: trn_boot §6 sets it at jax init, concourse
# bass_utils reads it when trace=True. The container's antenv package lacks
# this submodule, so provide it before anything imports jax/concourse.
if "antenv.axon_hooks" not in sys.modules:
    _ah = types.ModuleType("antenv.axon_hooks")
    _ah._hook = None

    def _set_hook(hook):
        _ah._hook = hook

    def _get_hook():
        return _ah._hook

    _ah.set_axon_ntff_profile_hook = _set_hook
    _ah.get_axon_ntff_profile_hook = _get_hook
    sys.modules["antenv.axon_hooks"] = _ah

import ml_dtypes  # noqa: E402
import concourse.bass as bass  # noqa: E402
import concourse.bacc as bacc  # noqa: E402
import concourse.tile as tile  # noqa: E402
from concourse import mybir  # noqa: E402
from concourse import bass_utils  # noqa: E402
from concourse.masks import make_identity  # noqa: E402

H = 0.5
EPS = 1e-8
B, D, R, O = 4096, 256, 128, 64
NCORES = 8
BL = B // NCORES          # 512 batch rows per core
NT = BL // 128            # 4 partition tiles per core
RO = R * O                # 8192
HO = O // 2               # 32 o's per half
HW = HO * R               # 4096 cols per half
F32 = mybir.dt.float32
BF16 = mybir.dt.bfloat16

_CACHE = {}


def _build():
    nc = bacc.Bacc("TRN2", target_bir_lowering=False, debug=False)
    X = nc.dram_tensor("X", [BL, D], F32, kind="ExternalInput")
    A = nc.dram_tensor("A", [D, R], BF16, kind="ExternalInput")
    Bm = nc.dram_tensor("Bm", [D, R], BF16, kind="ExternalInput")
    K2 = nc.dram_tensor("K2", [1, R], F32, kind="ExternalInput")
    W2p = nc.dram_tensor("W2p", [R, O], BF16, kind="ExternalInput")
    Wt = nc.dram_tensor("Wt", [D, RO], BF16, kind="ExternalInput")
    out = nc.dram_tensor("out", [BL, O], F32, kind="ExternalOutput")

    with tile.TileContext(nc) as tc, ExitStack() as ctx:
        consts = ctx.enter_context(tc.tile_pool(name="consts", bufs=1))
        xtp = ctx.enter_context(tc.tile_pool(name="xtp", bufs=4))
        tlp = ctx.enter_context(tc.tile_pool(name="tlp", bufs=4))
        work = ctx.enter_context(tc.tile_pool(name="work", bufs=2))
        gmp = ctx.enter_context(tc.tile_pool(name="gm", bufs=2))
        treep = ctx.enter_context(tc.tile_pool(name="tree", bufs=2))
        ps_x = ctx.enter_context(tc.tile_pool(name="ps_x", bufs=1, space="PSUM"))
        ps_m = ctx.enter_context(tc.tile_pool(name="ps_m", bufs=1, space="PSUM"))
        ps_e = ctx.enter_context(tc.tile_pool(name="ps_e", bufs=1, space="PSUM"))
        ps_o = ctx.enter_context(tc.tile_pool(name="ps_o", bufs=1, space="PSUM"))
        ps_g = ctx.enter_context(tc.tile_pool(name="ps_g", bufs=2, space="PSUM"))

        # ---- X tiles first on the sync queue (small, unblocks compute),
        # then the other small constants; Wt streams on SWDGE queues in
        # consumption order (quarter-major) ----
        xts = []
        for t in range(NT):
            xt = xtp.tile([128, D], F32, tag="xt", name=f"xt{t}")
            nc.sync.dma_start(out=xt[:, :], in_=X[t * 128:(t + 1) * 128, :])
            xts.append(xt)
        a_sb = consts.tile([128, 2, R], BF16, tag="a")
        bm_sb = consts.tile([128, 2, R], BF16, tag="bm")
        for c in range(2):
            nc.sync.dma_start(out=a_sb[:, c, :], in_=A[c * 128:(c + 1) * 128, :])
            nc.sync.dma_start(out=bm_sb[:, c, :], in_=Bm[c * 128:(c + 1) * 128, :])
        k2_sb = consts.tile([1, R], F32, tag="k2")
        nc.sync.dma_start(out=k2_sb[:, :], in_=K2[:, :])
        w2p_sb = consts.tile([R, O], BF16, tag="w2p")
        nc.sync.dma_start(out=w2p_sb[:, :], in_=W2p[:, :])
        identF = consts.tile([128, 128], F32, tag="idf")
        make_identity(nc, identF)
        identB = consts.tile([128, 128], BF16, tag="idb")
        make_identity(nc, identB)
        ones_sb = consts.tile([1, 128], F32, tag="ones")
        nc.vector.memset(ones_sb, 1.0)
        wt_sb = [[None] * 4, [None] * 4]
        for q in range(4):
            for c in range(2):
                t_ = consts.tile([128, 2048], BF16, tag=f"wt{c}{q}",
                                 name=f"wt{c}{q}")
                nc.gpsimd.dma_start(
                    out=t_[:, :], in_=Wt[c * 128:(c + 1) * 128, q * 2048:(q + 1) * 2048]
                )
                wt_sb[c][q] = t_

        # ---- prologue for all tiles: transpose, membership, softmax, out2 ----
        xTbs, e_ns, out2s = [], [], []
        for t in range(NT):
            xtT_ps = ps_x.tile([128, D], F32, tag="xtT")
            for c in range(2):
                nc.tensor.transpose(
                    xtT_ps[:, c * 128:(c + 1) * 128],
                    xts[t][:, c * 128:(c + 1) * 128],
                    identF,
                )
            x2b = work.tile([128, D], BF16, tag="x2b")
            xTb = tlp.tile([128, D], BF16, tag="xTb", name=f"xTb{t}")
            nc.scalar.activation(x2b, xtT_ps, mybir.ActivationFunctionType.Square)
            nc.vector.tensor_copy(xTb, xtT_ps)

            m_ps = ps_m.tile([128, R], F32, tag="m")
            nc.tensor.matmul(m_ps, lhsT=x2b[:, 0:128], rhs=a_sb[:, 0, :],
                             start=True, stop=False)
            nc.tensor.matmul(m_ps, lhsT=x2b[:, 128:256], rhs=a_sb[:, 1, :],
                             start=False, stop=False)
            nc.tensor.matmul(m_ps, lhsT=xTb[:, 0:128], rhs=bm_sb[:, 0, :],
                             start=False, stop=False)
            nc.tensor.matmul(m_ps, lhsT=xTb[:, 128:256], rhs=bm_sb[:, 1, :],
                             start=False, stop=False)
            nc.tensor.matmul(m_ps, lhsT=ones_sb, rhs=k2_sb,
                             start=False, stop=True)

            nmx = work.tile([128, 1], F32, tag="nmx")
            nc.vector.reduce_max(nmx, m_ps, axis=mybir.AxisListType.X, negate=True)
            e_bf = work.tile([128, R], BF16, tag="e")
            s_ = work.tile([128, 1], F32, tag="s")
            nc.scalar.activation(e_bf, m_ps, mybir.ActivationFunctionType.Exp,
                                 bias=nmx, scale=1.0, accum_out=s_)
            rs = work.tile([128, 1], F32, tag="rs")
            nc.vector.reciprocal(rs, s_)
            e_n = tlp.tile([128, R], BF16, tag="en", name=f"en{t}")
            nc.vector.tensor_scalar_mul(e_n, e_bf, rs)

            eT_ps = ps_e.tile([128, 128], BF16, tag="eT")
            nc.tensor.transpose(eT_ps, e_n, identB)
            eT_sb = work.tile([128, 128], BF16, tag="eTsb")
            nc.scalar.copy(eT_sb, eT_ps)
            out2_ps = ps_o.tile([128, O], F32, tag="out2")
            nc.tensor.matmul(out2_ps, lhsT=eT_sb, rhs=w2p_sb, start=True, stop=True)
            out2_sb = tlp.tile([128, O], F32, tag="o2sb", name=f"o2sb{t}")
            nc.vector.tensor_copy(out2_sb, out2_ps)

            xTbs.append(xTb)
            e_ns.append(e_n)
            out2s.append(out2_sb)

        # ---- G phases: per tile, two o-halves of X @ Wt;
        # Scalar evicts PSUM->bf16, DVE multiplies by e and tree-reduces ----
        for t in range(NT):
            xTb, e_n = xTbs[t], e_ns[t]
            osb = work.tile([128, O], F32, tag="osb", name=f"osb{t}")
            for h in range(2):
                gm = gmp.tile([128, HW], BF16, tag=f"gm{h}", name=f"gm_{t}_{h}")
                for hc in range(4):
                    ch = h * 4 + hc
                    gt = ps_g.tile([128, 1024], F32, tag="g", name=f"g_{t}_{ch}")
                    for c in range(2):
                        for half in range(2):
                            nch = 2 * ch + half
                            q, col = divmod(nch * 512, 2048)
                            nc.tensor.matmul(
                                gt[:, half * 512:(half + 1) * 512],
                                lhsT=xTb[:, c * 128:(c + 1) * 128],
                                rhs=wt_sb[c][q][:, col:col + 512],
                                start=(c == 0), stop=(c == 1),
                            )
                    nc.scalar.copy(gm[:, hc * 1024:(hc + 1) * 1024], gt)

                gv = gm.rearrange("p (o r) -> p o r", o=HO)
                ebc = e_n.rearrange("p r -> p () r").broadcast_to((128, HO, R))
                nc.vector.tensor_mul(gv, gv, ebc)

                prev = gm
                r = R
                while r > 2:
                    nxt = treep.tile([128, HO * (r // 2)], BF16,
                                     tag=f"tr{h}{r}", name=f"tr_{t}_{h}_{r}")
                    pv = prev.rearrange("p (o r) -> p o r", o=HO)
                    nv = nxt.rearrange("p (o r) -> p o r", o=HO)
                    nc.vector.tensor_add(nv, pv[:, :, 0:r // 2], pv[:, :, r // 2:r])
                    prev = nxt
                    r //= 2
                pv = prev.rearrange("p (o r) -> p o r", o=HO)
                hsl = slice(h * HO, (h + 1) * HO)
                red = work.tile([128, HO], F32, tag=f"red{h}", name=f"red_{t}_{h}")
                nc.vector.tensor_add(red.rearrange("p o -> p o ()"),
                                     pv[:, :, 0:1], pv[:, :, 1:2])
                nc.vector.tensor_add(osb[:, hsl], red, out2s[t][:, hsl])
            nc.sync.dma_start(out=out[t * 128:(t + 1) * 128, :], in_=osb)

    nc.finalize()
    return nc


def _get_nc():
    if "nc" not in _CACHE:
        _CACHE["nc"] = _build()
    return _CACHE["nc"]


def _host_prep(centers, sigmas, W, b):
    c64 = centers.astype(np.float64)
    S = (H / sigmas.astype(np.float64) ** 2) + EPS          # (D,R)
    A = (-S / D).astype(ml_dtypes.bfloat16)                  # X^2 coeff
    Bm = (2.0 * S * c64 / D).astype(ml_dtypes.bfloat16)      # X coeff
    K2 = (-(S * c64 * c64).sum(axis=0, keepdims=True) / D).astype(np.float32)
    W1 = W[: D * R].reshape(R, D, O)
    # o-major: Wt[d, o*R + r] = W1[r, d, o]
    Wt = np.ascontiguousarray(W1.transpose(1, 2, 0).reshape(D, RO)).astype(
        ml_dtypes.bfloat16
    )
    W2p = (W[D * R:].astype(np.float64) + b[None, :].astype(np.float64)).astype(
        ml_dtypes.bfloat16
    )
    return A, Bm, K2, W2p, Wt


def kernel(X, centers, sigmas, W, b):
    X = np.asarray(X, dtype=np.float32)
    centers = np.asarray(centers, dtype=np.float32)
    sigmas = np.asarray(sigmas, dtype=np.float32)
    W = np.asarray(W, dtype=np.float32)
    b = np.asarray(b, dtype=np.float32)

    A, Bm, K2, W2p, Wt = _host_prep(centers, sigmas, W, b)
    nc = _get_nc()
    in_maps = [
        {
            "X": np.ascontiguousarray(X[k * BL:(k + 1) * BL]),
            "A": A, "Bm": Bm, "K2": K2, "W2p": W2p, "Wt": Wt,
        }
        for k in range(NCORES)
    ]
    res = bass_utils.run_bass_kernel_spmd(nc, in_maps, core_ids=list(range(NCORES)))
    return np.concatenate([res.results[k]["out"] for k in range(NCORES)], axis=0)


# revision 10
# speedup vs baseline: 1.2833x; 1.0052x over previous
"""HTSK fuzzy-system kernel for Trainium2 (Bass/Tile), 8-core data-parallel.

Math (per batch row b):
  S     = H/sigma^2 + EPS                          (D,R)
  m     = mean_d(-(X_bd - C_dr)^2 * S_dr)          (B,R)
        = X^2 @ (-S/D) + X @ (2*S*C/D) + K2        (matmul expansion)
  e     = exp(m - max_r m) / sum_r exp(...)        (normalized firing)
  out   = sum_r e_br * G_bro  +  e @ (W2 + 1 b^T)
  G     = X @ Wt,  Wt[d, o*R+r] = W[r*D+d, o]      (B, O*R)  o-major

o-major G columns make the firing-strength multiply read e with a
contiguous innermost r-run (DVE 2x mode) and give the r-reduction tree
strided-outer / flat-write access patterns that also keep 2x mode.

Schedule: all four per-tile prologues (transpose, membership, softmax,
out2) run first — they only need the small DMAs — hiding the ~12 us Wt
stream; then the four G phases run back-to-back (PE dense, Scalar
evicts PSUM, DVE multiplies + reduces, in two o-halves per tile).

Sharding: batch B=4096 split 512 rows per core; weights replicated.
"""
import sys
import types
from contextlib import ExitStack

import numpy as np

sys.path.insert(0, "/opt/trn_rl_repo")

# NTFF profile-hook registry: trn_boot §6 sets it at jax init, concourse
# bass_utils reads it when trace=True. The container's antenv package lacks
# this submodule, so provide it before anything imports jax/concourse.
if "antenv.axon_hooks" not in sys.modules:
    _ah = types.ModuleType("antenv.axon_hooks")
    _ah._hook = None

    def _set_hook(hook):
        _ah._hook = hook

    def _get_hook():
        return _ah._hook

    _ah.set_axon_ntff_profile_hook = _set_hook
    _ah.get_axon_ntff_profile_hook = _get_hook
    sys.modules["antenv.axon_hooks"] = _ah

import ml_dtypes  # noqa: E402
import concourse.bass as bass  # noqa: E402
import concourse.bacc as bacc  # noqa: E402
import concourse.tile as tile  # noqa: E402
from concourse import mybir  # noqa: E402
from concourse import bass_utils  # noqa: E402
from concourse.masks import make_identity  # noqa: E402

H = 0.5
EPS = 1e-8
B, D, R, O = 4096, 256, 128, 64
NCORES = 8
BL = B // NCORES          # 512 batch rows per core
NT = BL // 128            # 4 partition tiles per core
RO = R * O                # 8192
HO = O // 2               # 32 o's per half
HW = HO * R               # 4096 cols per half
F32 = mybir.dt.float32
BF16 = mybir.dt.bfloat16

_CACHE = {}


def _build():
    nc = bacc.Bacc("TRN2", target_bir_lowering=False, debug=False)
    X = nc.dram_tensor("X", [BL, D], F32, kind="ExternalInput")
    A = nc.dram_tensor("A", [D, R], BF16, kind="ExternalInput")
    Bm = nc.dram_tensor("Bm", [D, R], BF16, kind="ExternalInput")
    K2 = nc.dram_tensor("K2", [1, R], F32, kind="ExternalInput")
    W2p = nc.dram_tensor("W2p", [R, O], BF16, kind="ExternalInput")
    Wt = nc.dram_tensor("Wt", [D, RO], BF16, kind="ExternalInput")
    out = nc.dram_tensor("out", [BL, O], F32, kind="ExternalOutput")

    with tile.TileContext(nc) as tc, ExitStack() as ctx:
        consts = ctx.enter_context(tc.tile_pool(name="consts", bufs=1))
        xtp = ctx.enter_context(tc.tile_pool(name="xtp", bufs=4))
        tlp = ctx.enter_context(tc.tile_pool(name="tlp", bufs=4))
        work = ctx.enter_context(tc.tile_pool(name="work", bufs=2))
        gmp = ctx.enter_context(tc.tile_pool(name="gm", bufs=2))
        treep = ctx.enter_context(tc.tile_pool(name="tree", bufs=2))
        ps_x = ctx.enter_context(tc.tile_pool(name="ps_x", bufs=1, space="PSUM"))
        ps_m = ctx.enter_context(tc.tile_pool(name="ps_m", bufs=1, space="PSUM"))
        ps_e = ctx.enter_context(tc.tile_pool(name="ps_e", bufs=1, space="PSUM"))
        ps_o = ctx.enter_context(tc.tile_pool(name="ps_o", bufs=1, space="PSUM"))
        ps_g = ctx.enter_context(tc.tile_pool(name="ps_g", bufs=2, space="PSUM"))

        # ---- X tiles first on the sync queue (small, unblocks compute),
        # then the other small constants; Wt streams on SWDGE queues in
        # consumption order (quarter-major) ----
        a_sb = consts.tile([128, 2, R], BF16, tag="a")
        bm_sb = consts.tile([128, 2, R], BF16, tag="bm")
        for c in range(2):
            nc.sync.dma_start(out=a_sb[:, c, :], in_=A[c * 128:(c + 1) * 128, :])
            nc.sync.dma_start(out=bm_sb[:, c, :], in_=Bm[c * 128:(c + 1) * 128, :])
        k2_sb = consts.tile([1, R], F32, tag="k2")
        nc.sync.dma_start(out=k2_sb[:, :], in_=K2[:, :])
        w2p_sb = consts.tile([R, O], BF16, tag="w2p")
        nc.sync.dma_start(out=w2p_sb[:, :], in_=W2p[:, :])
        xts = []
        for t in range(NT):
            xt = xtp.tile([128, D], F32, tag="xt", name=f"xt{t}")
            nc.sync.dma_start(out=xt[:, :], in_=X[t * 128:(t + 1) * 128, :])
            xts.append(xt)
        identF = consts.tile([128, 128], F32, tag="idf")
        make_identity(nc, identF)
        identB = consts.tile([128, 128], BF16, tag="idb")
        make_identity(nc, identB)
        ones_sb = consts.tile([1, 128], F32, tag="ones")
        nc.vector.memset(ones_sb, 1.0)
        wt_sb = [[None] * 4, [None] * 4]
        for q in range(4):
            for c in range(2):
                t_ = consts.tile([128, 2048], BF16, tag=f"wt{c}{q}",
                                 name=f"wt{c}{q}")
                nc.gpsimd.dma_start(
                    out=t_[:, :], in_=Wt[c * 128:(c + 1) * 128, q * 2048:(q + 1) * 2048]
                )
                wt_sb[c][q] = t_

        # ---- prologue for all tiles: transpose, membership, softmax, out2 ----
        xTbs, e_ns, out2s = [], [], []
        for t in range(NT):
            xtT_ps = ps_x.tile([128, D], F32, tag="xtT")
            for c in range(2):
                nc.tensor.transpose(
                    xtT_ps[:, c * 128:(c + 1) * 128],
                    xts[t][:, c * 128:(c + 1) * 128],
                    identF,
                )
            x2b = work.tile([128, D], BF16, tag="x2b")
            xTb = tlp.tile([128, D], BF16, tag="xTb", name=f"xTb{t}")
            nc.scalar.activation(x2b, xtT_ps, mybir.ActivationFunctionType.Square)
            nc.vector.tensor_copy(xTb, xtT_ps)

            m_ps = ps_m.tile([128, R], F32, tag="m")
            nc.tensor.matmul(m_ps, lhsT=x2b[:, 0:128], rhs=a_sb[:, 0, :],
                             start=True, stop=False)
            nc.tensor.matmul(m_ps, lhsT=x2b[:, 128:256], rhs=a_sb[:, 1, :],
                             start=False, stop=False)
            nc.tensor.matmul(m_ps, lhsT=xTb[:, 0:128], rhs=bm_sb[:, 0, :],
                             start=False, stop=False)
            nc.tensor.matmul(m_ps, lhsT=xTb[:, 128:256], rhs=bm_sb[:, 1, :],
                             start=False, stop=False)
            nc.tensor.matmul(m_ps, lhsT=ones_sb, rhs=k2_sb,
                             start=False, stop=True)

            nmx = work.tile([128, 1], F32, tag="nmx")
            nc.vector.reduce_max(nmx, m_ps, axis=mybir.AxisListType.X, negate=True)
            e_bf = work.tile([128, R], BF16, tag="e")
            s_ = work.tile([128, 1], F32, tag="s")
            nc.scalar.activation(e_bf, m_ps, mybir.ActivationFunctionType.Exp,
                                 bias=nmx, scale=1.0, accum_out=s_)
            rs = work.tile([128, 1], F32, tag="rs")
            nc.vector.reciprocal(rs, s_)
            e_n = tlp.tile([128, R], BF16, tag="en", name=f"en{t}")
            nc.vector.tensor_scalar_mul(e_n, e_bf, rs)

            eT_ps = ps_e.tile([128, 128], BF16, tag="eT")
            nc.tensor.transpose(eT_ps, e_n, identB)
            eT_sb = work.tile([128, 128], BF16, tag="eTsb")
            nc.scalar.copy(eT_sb, eT_ps)
            out2_ps = ps_o.tile([128, O], F32, tag="out2")
            nc.tensor.matmul(out2_ps, lhsT=eT_sb, rhs=w2p_sb, start=True, stop=True)
            out2_sb = tlp.tile([128, O], F32, tag="o2sb", name=f"o2sb{t}")
            nc.vector.tensor_copy(out2_sb, out2_ps)

            xTbs.append(xTb)
            e_ns.append(e_n)
            out2s.append(out2_sb)

        # ---- G phases: per tile, two o-halves of X @ Wt;
        # Scalar evicts PSUM->bf16, DVE multiplies by e and tree-reduces ----
        for t in range(NT):
            xTb, e_n = xTbs[t], e_ns[t]
            osb = work.tile([128, O], F32, tag="osb", name=f"osb{t}")
            for h in range(2):
                gm = gmp.tile([128, HW], BF16, tag=f"gm{h}", name=f"gm_{t}_{h}")
                for hc in range(4):
                    ch = h * 4 + hc
                    gt = ps_g.tile([128, 1024], F32, tag="g", name=f"g_{t}_{ch}")
                    for c in range(2):
                        for half in range(2):
                            nch = 2 * ch + half
                            q, col = divmod(nch * 512, 2048)
                            nc.tensor.matmul(
                                gt[:, half * 512:(half + 1) * 512],
                                lhsT=xTb[:, c * 128:(c + 1) * 128],
                                rhs=wt_sb[c][q][:, col:col + 512],
                                start=(c == 0), stop=(c == 1),
                            )
                    nc.scalar.copy(gm[:, hc * 1024:(hc + 1) * 1024], gt)

                gv = gm.rearrange("p (o r) -> p o r", o=HO)
                ebc = e_n.rearrange("p r -> p () r").broadcast_to((128, HO, R))
                nc.vector.tensor_mul(gv, gv, ebc)

                prev = gm
                r = R
                while r > 2:
                    nxt = treep.tile([128, HO * (r // 2)], BF16,
                                     tag=f"tr{h}{r}", name=f"tr_{t}_{h}_{r}")
                    pv = prev.rearrange("p (o r) -> p o r", o=HO)
                    nv = nxt.rearrange("p (o r) -> p o r", o=HO)
                    nc.vector.tensor_add(nv, pv[:, :, 0:r // 2], pv[:, :, r // 2:r])
                    prev = nxt
                    r //= 2
                pv = prev.rearrange("p (o r) -> p o r", o=HO)
                hsl = slice(h * HO, (h + 1) * HO)
                red = work.tile([128, HO], F32, tag=f"red{h}", name=f"red_{t}_{h}")
                nc.vector.tensor_add(red.rearrange("p o -> p o ()"),
                                     pv[:, :, 0:1], pv[:, :, 1:2])
                nc.vector.tensor_add(osb[:, hsl], red, out2s[t][:, hsl])
            nc.sync.dma_start(out=out[t * 128:(t + 1) * 128, :], in_=osb)

    nc.finalize()
    return nc


def _get_nc():
    if "nc" not in _CACHE:
        _CACHE["nc"] = _build()
    return _CACHE["nc"]


def _host_prep(centers, sigmas, W, b):
    c64 = centers.astype(np.float64)
    S = (H / sigmas.astype(np.float64) ** 2) + EPS          # (D,R)
    A = (-S / D).astype(ml_dtypes.bfloat16)                  # X^2 coeff
    Bm = (2.0 * S * c64 / D).astype(ml_dtypes.bfloat16)      # X coeff
    K2 = (-(S * c64 * c64).sum(axis=0, keepdims=True) / D).astype(np.float32)
    W1 = W[: D * R].reshape(R, D, O)
    # o-major: Wt[d, o*R + r] = W1[r, d, o]
    Wt = np.ascontiguousarray(W1.transpose(1, 2, 0).reshape(D, RO)).astype(
        ml_dtypes.bfloat16
    )
    W2p = (W[D * R:].astype(np.float64) + b[None, :].astype(np.float64)).astype(
        ml_dtypes.bfloat16
    )
    return A, Bm, K2, W2p, Wt


def kernel(X, centers, sigmas, W, b):
    X = np.asarray(X, dtype=np.float32)
    centers = np.asarray(centers, dtype=np.float32)
    sigmas = np.asarray(sigmas, dtype=np.float32)
    W = np.asarray(W, dtype=np.float32)
    b = np.asarray(b, dtype=np.float32)

    A, Bm, K2, W2p, Wt = _host_prep(centers, sigmas, W, b)
    nc = _get_nc()
    in_maps = [
        {
            "X": np.ascontiguousarray(X[k * BL:(k + 1) * BL]),
            "A": A, "Bm": Bm, "K2": K2, "W2p": W2p, "Wt": Wt,
        }
        for k in range(NCORES)
    ]
    res = bass_utils.run_bass_kernel_spmd(nc, in_maps, core_ids=list(range(NCORES)))
    return np.concatenate([res.results[k]["out"] for k in range(NCORES)], axis=0)


# revision 12
# speedup vs baseline: 1.3378x; 1.0424x over previous
"""HTSK fuzzy-system kernel for Trainium2 (Bass/Tile), 8-core data-parallel.

Math (per batch row b):
  S     = H/sigma^2 + EPS                          (D,R)
  m     = mean_d(-(X_bd - C_dr)^2 * S_dr)          (B,R)
        = X^2 @ (-S/D) + X @ (2*S*C/D) + K2        (matmul expansion)
  e     = exp(m - max_r m) / sum_r exp(...)        (normalized firing)
  out   = sum_r e_br * G_bro  +  e @ (W2 + 1 b^T)
  G     = X @ Wt,  Wt[d, o*R+r] = W[r*D+d, o]      (B, O*R)  o-major

o-major G columns make the firing-strength multiply read e with a
contiguous innermost r-run (DVE 2x mode) and give the r-reduction tree
strided-outer / flat-write access patterns that also keep 2x mode.

Schedule: all four per-tile prologues (transpose, membership, softmax,
out2) run first — they only need the small DMAs — hiding the ~12 us Wt
stream; then the four G phases run back-to-back (PE dense, Scalar
evicts PSUM, DVE multiplies + reduces, in two o-halves per tile).

Sharding: batch B=4096 split 512 rows per core; weights replicated.
"""
import sys
import types
from contextlib import ExitStack

import numpy as np

sys.path.insert(0, "/opt/trn_rl_repo")

# NTFF profile-hook registry: trn_boot §6 sets it at jax init, concourse
# bass_utils reads it when trace=True. The container's antenv package lacks
# this submodule, so provide it before anything imports jax/concourse.
if "antenv.axon_hooks" not in sys.modules:
    _ah = types.ModuleType("antenv.axon_hooks")
    _ah._hook = None

    def _set_hook(hook):
        _ah._hook = hook

    def _get_hook():
        return _ah._hook

    _ah.set_axon_ntff_profile_hook = _set_hook
    _ah.get_axon_ntff_profile_hook = _get_hook
    sys.modules["antenv.axon_hooks"] = _ah

import ml_dtypes  # noqa: E402
import concourse.bass as bass  # noqa: E402
import concourse.bacc as bacc  # noqa: E402
import concourse.tile as tile  # noqa: E402
from concourse import mybir  # noqa: E402
from concourse import bass_utils  # noqa: E402
from concourse.masks import make_identity  # noqa: E402

H = 0.5
EPS = 1e-8
B, D, R, O = 4096, 256, 128, 64
NCORES = 8
BL = B // NCORES          # 512 batch rows per core
NT = BL // 128            # 4 partition tiles per core
RO = R * O                # 8192
HO = O // 2               # 32 o's per half
HW = HO * R               # 4096 cols per half
F32 = mybir.dt.float32
BF16 = mybir.dt.bfloat16

_CACHE = {}


def _build():
    nc = bacc.Bacc("TRN2", target_bir_lowering=False, debug=False)
    X = nc.dram_tensor("X", [BL, D], F32, kind="ExternalInput")
    A = nc.dram_tensor("A", [D, R], BF16, kind="ExternalInput")
    Bm = nc.dram_tensor("Bm", [D, R], BF16, kind="ExternalInput")
    K2 = nc.dram_tensor("K2", [1, R], F32, kind="ExternalInput")
    W2p = nc.dram_tensor("W2p", [R, O], BF16, kind="ExternalInput")
    Wt = nc.dram_tensor("Wt", [D, RO], BF16, kind="ExternalInput")
    out = nc.dram_tensor("out", [BL, O], F32, kind="ExternalOutput")

    with tile.TileContext(nc) as tc, ExitStack() as ctx:
        consts = ctx.enter_context(tc.tile_pool(name="consts", bufs=1))
        xtp = ctx.enter_context(tc.tile_pool(name="xtp", bufs=4))
        tlp = ctx.enter_context(tc.tile_pool(name="tlp", bufs=4))
        work = ctx.enter_context(tc.tile_pool(name="work", bufs=2))
        gmp = ctx.enter_context(tc.tile_pool(name="gm", bufs=2))
        treep = ctx.enter_context(tc.tile_pool(name="tree", bufs=2))
        ps_x = ctx.enter_context(tc.tile_pool(name="ps_x", bufs=1, space="PSUM"))
        ps_m = ctx.enter_context(tc.tile_pool(name="ps_m", bufs=1, space="PSUM"))
        ps_e = ctx.enter_context(tc.tile_pool(name="ps_e", bufs=1, space="PSUM"))
        ps_o = ctx.enter_context(tc.tile_pool(name="ps_o", bufs=1, space="PSUM"))
        ps_g = ctx.enter_context(tc.tile_pool(name="ps_g", bufs=2, space="PSUM"))

        # ---- X tiles first on the sync queue (small, unblocks compute),
        # then the other small constants; Wt streams on SWDGE queues in
        # consumption order (quarter-major) ----
        xts = []
        xt0 = xtp.tile([128, D], F32, tag="xt", name="xt0")
        nc.sync.dma_start(out=xt0[:, :], in_=X[0:128, :])
        xts.append(xt0)
        a_sb = consts.tile([128, 2, R], BF16, tag="a")
        bm_sb = consts.tile([128, 2, R], BF16, tag="bm")
        for c in range(2):
            nc.sync.dma_start(out=a_sb[:, c, :], in_=A[c * 128:(c + 1) * 128, :])
            nc.sync.dma_start(out=bm_sb[:, c, :], in_=Bm[c * 128:(c + 1) * 128, :])
        k2_sb = consts.tile([1, R], F32, tag="k2")
        nc.sync.dma_start(out=k2_sb[:, :], in_=K2[:, :])
        for t in range(1, NT):
            xt = xtp.tile([128, D], F32, tag="xt", name=f"xt{t}")
            nc.sync.dma_start(out=xt[:, :], in_=X[t * 128:(t + 1) * 128, :])
            xts.append(xt)
        w2p_sb = consts.tile([R, O], BF16, tag="w2p")
        nc.sync.dma_start(out=w2p_sb[:, :], in_=W2p[:, :])
        identF = consts.tile([128, 128], F32, tag="idf")
        make_identity(nc, identF)
        identB = consts.tile([128, 128], BF16, tag="idb")
        make_identity(nc, identB)
        ones_sb = consts.tile([1, 128], F32, tag="ones")
        nc.vector.memset(ones_sb, 1.0)
        # gate the big Wt stream behind X0's arrival so the small transfers
        # win the HBM race (the GpSimd queue is FIFO: this copy waits on the
        # X0 DMA, holding back the Wt descriptor pushes behind it)
        gate = consts.tile([1, 4], F32, tag="gate")
        nc.gpsimd.tensor_copy(gate, xt0[0:1, 0:4])
        wt_sb = [[None] * 4, [None] * 4]
        for q in range(4):
            for c in range(2):
                t_ = consts.tile([128, 2048], BF16, tag=f"wt{c}{q}",
                                 name=f"wt{c}{q}")
                nc.gpsimd.dma_start(
                    out=t_[:, :], in_=Wt[c * 128:(c + 1) * 128, q * 2048:(q + 1) * 2048]
                )
                wt_sb[c][q] = t_

        # ---- prologue for all tiles: transpose, membership, softmax, out2 ----
        xTbs, e_ns, out2s = [], [], []
        for t in range(NT):
            xtT_ps = ps_x.tile([128, D], F32, tag="xtT")
            for c in range(2):
                nc.tensor.transpose(
                    xtT_ps[:, c * 128:(c + 1) * 128],
                    xts[t][:, c * 128:(c + 1) * 128],
                    identF,
                )
            x2b = work.tile([128, D], BF16, tag="x2b")
            xTb = tlp.tile([128, D], BF16, tag="xTb", name=f"xTb{t}")
            nc.scalar.activation(x2b, xtT_ps, mybir.ActivationFunctionType.Square)
            nc.vector.tensor_copy(xTb, xtT_ps)

            m_ps = ps_m.tile([128, R], F32, tag="m")
            nc.tensor.matmul(m_ps, lhsT=x2b[:, 0:128], rhs=a_sb[:, 0, :],
                             start=True, stop=False)
            nc.tensor.matmul(m_ps, lhsT=x2b[:, 128:256], rhs=a_sb[:, 1, :],
                             start=False, stop=False)
            nc.tensor.matmul(m_ps, lhsT=xTb[:, 0:128], rhs=bm_sb[:, 0, :],
                             start=False, stop=False)
            nc.tensor.matmul(m_ps, lhsT=xTb[:, 128:256], rhs=bm_sb[:, 1, :],
                             start=False, stop=False)
            nc.tensor.matmul(m_ps, lhsT=ones_sb, rhs=k2_sb,
                             start=False, stop=True)

            nmx = work.tile([128, 1], F32, tag="nmx")
            nc.vector.reduce_max(nmx, m_ps, axis=mybir.AxisListType.X, negate=True)
            e_bf = work.tile([128, R], BF16, tag="e")
            s_ = work.tile([128, 1], F32, tag="s")
            nc.scalar.activation(e_bf, m_ps, mybir.ActivationFunctionType.Exp,
                                 bias=nmx, scale=1.0, accum_out=s_)
            rs = work.tile([128, 1], F32, tag="rs")
            nc.vector.reciprocal(rs, s_)
            e_n = tlp.tile([128, R], BF16, tag="en", name=f"en{t}")
            nc.vector.tensor_scalar_mul(e_n, e_bf, rs)

            eT_ps = ps_e.tile([128, 128], BF16, tag="eT")
            nc.tensor.transpose(eT_ps, e_n, identB)
            eT_sb = work.tile([128, 128], BF16, tag="eTsb")
            nc.scalar.copy(eT_sb, eT_ps)
            out2_ps = ps_o.tile([128, O], F32, tag="out2")
            nc.tensor.matmul(out2_ps, lhsT=eT_sb, rhs=w2p_sb, start=True, stop=True)
            out2_sb = tlp.tile([128, O], F32, tag="o2sb", name=f"o2sb{t}")
            nc.vector.tensor_copy(out2_sb, out2_ps)

            xTbs.append(xTb)
            e_ns.append(e_n)
            out2s.append(out2_sb)

        # ---- G phases: per tile, two o-halves of X @ Wt;
        # Scalar evicts PSUM->bf16, DVE multiplies by e and tree-reduces ----
        for t in range(NT):
            xTb, e_n = xTbs[t], e_ns[t]
            osb = work.tile([128, O], F32, tag="osb", name=f"osb{t}")
            for h in range(2):
                gm = gmp.tile([128, HW], BF16, tag=f"gm{h}", name=f"gm_{t}_{h}")
                for hc in range(4):
                    ch = h * 4 + hc
                    gt = ps_g.tile([128, 1024], F32, tag="g", name=f"g_{t}_{ch}")
                    for c in range(2):
                        for half in range(2):
                            nch = 2 * ch + half
                            q, col = divmod(nch * 512, 2048)
                            nc.tensor.matmul(
                                gt[:, half * 512:(half + 1) * 512],
                                lhsT=xTb[:, c * 128:(c + 1) * 128],
                                rhs=wt_sb[c][q][:, col:col + 512],
                                start=(c == 0), stop=(c == 1),
                            )
                    nc.scalar.copy(gm[:, hc * 1024:(hc + 1) * 1024], gt)

                gv = gm.rearrange("p (o r) -> p o r", o=HO)
                ebc = e_n.rearrange("p r -> p () r").broadcast_to((128, HO, R))
                nc.vector.tensor_mul(gv, gv, ebc)

                prev = gm
                r = R
                while r > 2:
                    nxt = treep.tile([128, HO * (r // 2)], BF16,
                                     tag=f"tr{h}{r}", name=f"tr_{t}_{h}_{r}")
                    pv = prev.rearrange("p (o r) -> p o r", o=HO)
                    nv = nxt.rearrange("p (o r) -> p o r", o=HO)
                    nc.vector.tensor_add(nv, pv[:, :, 0:r // 2], pv[:, :, r // 2:r])
                    prev = nxt
                    r //= 2
                pv = prev.rearrange("p (o r) -> p o r", o=HO)
                hsl = slice(h * HO, (h + 1) * HO)
                red = work.tile([128, HO], F32, tag=f"red{h}", name=f"red_{t}_{h}")
                nc.vector.tensor_add(red.rearrange("p o -> p o ()"),
                                     pv[:, :, 0:1], pv[:, :, 1:2])
                nc.vector.tensor_add(osb[:, hsl], red, out2s[t][:, hsl])
            nc.sync.dma_start(out=out[t * 128:(t + 1) * 128, :], in_=osb)

    nc.finalize()
    return nc


def _get_nc():
    if "nc" not in _CACHE:
        _CACHE["nc"] = _build()
    return _CACHE["nc"]


def _host_prep(centers, sigmas, W, b):
    c64 = centers.astype(np.float64)
    S = (H / sigmas.astype(np.float64) ** 2) + EPS          # (D,R)
    A = (-S / D).astype(ml_dtypes.bfloat16)                  # X^2 coeff
    Bm = (2.0 * S * c64 / D).astype(ml_dtypes.bfloat16)      # X coeff
    K2 = (-(S * c64 * c64).sum(axis=0, keepdims=True) / D).astype(np.float32)
    W1 = W[: D * R].reshape(R, D, O)
    # o-major: Wt[d, o*R + r] = W1[r, d, o]
    Wt = np.ascontiguousarray(W1.transpose(1, 2, 0).reshape(D, RO)).astype(
        ml_dtypes.bfloat16
    )
    W2p = (W[D * R:].astype(np.float64) + b[None, :].astype(np.float64)).astype(
        ml_dtypes.bfloat16
    )
    return A, Bm, K2, W2p, Wt


def kernel(X, centers, sigmas, W, b):
    X = np.asarray(X, dtype=np.float32)
    centers = np.asarray(centers, dtype=np.float32)
    sigmas = np.asarray(sigmas, dtype=np.float32)
    W = np.asarray(W, dtype=np.float32)
    b = np.asarray(b, dtype=np.float32)

    A, Bm, K2, W2p, Wt = _host_prep(centers, sigmas, W, b)
    nc = _get_nc()
    in_maps = [
        {
            "X": np.ascontiguousarray(X[k * BL:(k + 1) * BL]),
            "A": A, "Bm": Bm, "K2": K2, "W2p": W2p, "Wt": Wt,
        }
        for k in range(NCORES)
    ]
    res = bass_utils.run_bass_kernel_spmd(nc, in_maps, core_ids=list(range(NCORES)))
    return np.concatenate([res.results[k]["out"] for k in range(NCORES)], axis=0)
